# revision 2
# baseline (speedup 1.0000x reference)
"""Trainium2 Bass kernel for nn_CLRBP_23124103922240.

Math: scores[s, c] = x[s] . W[c] + b[c], softmax over 16 classes, where
W[c] = g * tile4x4(A1[c]) + (1-g) * A2[c],
A1[c] = u1 u1^T - v1 v1^T (64x64, rank 8), A2[c] = u2 u2^T - v2 v2^T
(256x256, rank 2), g = sigmoid(l[0]).

Strategy (dense-W, X-stationary):
  - x is cast to fp16 on host (measured output rel-err 4.3e-3, gate 2e-2)
    and re-laid out per core as xt[p, q, s] = x[s, q*128 + p]: the flat
    65536-pixel axis is split into 512 chunks of 128; the DMA stream is
    16.8 MB/core, half of f32.
  - For each chunk q the X block [128 pixels, 128 samples] is the
    *stationary* matmul operand; two [128, 16] moving operands (the g*A1
    table slice and the (1-g)*W2 slice) accumulate scores [128 samples,
    16 classes] directly in PSUM.  No per-sample vector work at all.
  - W1-tiled never materializes: tile4x4 means the moving slice for chunk
    (m, nh) is g*A1[c, m%64, p%64], read from a host-precomputed
    [128, 64, 16] table (p-duplicated so no partition wrap is needed).
  - W2 (dense [65536, 16]) is generated on device from the rank-2 factors:
    for each row m and column half nh, out[p, c] = sum_k st2[k, nh*128+p]
    * dg[k, m, c] with st2 = [u2 | v2] and dg the per-m diag-packed
    (1-g)-scaled factors; 512 tiny matmuls + 16 PSUM->SBUF copies.
  - Bias enters as a K=1 matmul (ones x b); softmax on [128, 16] f32.

Data-parallel over 8 NeuronCores: batch 1024 -> 128 samples per core.
"""

import numpy as np

import concourse.bacc as bacc
import concourse.mybir as mybir
import concourse.tile as tile
from concourse.bass_utils import run_bass_kernel_spmd

N_CORES = 8
B, D, C = 1024, 256, 16
BL = B // N_CORES        # 128 samples per core
NQ = (D * D) // 128      # 512 pixel chunks
G = 16                   # chunks per x DMA group
NG = NQ // G             # 32 groups
MBLK = 32                # W2-gen rows per PSUM bank round

F16 = mybir.dt.float16
F32 = mybir.dt.float32
AOP = mybir.AluOpType
AFT = mybir.ActivationFunctionType
AXL = mybir.AxisListType

_cache = {}


def _build():
    if "nc" in _cache:
        return _cache["nc"]

    nc = bacc.Bacc("TRN2", target_bir_lowering=False, debug=False,
                   num_devices=N_CORES)

    xt_d = nc.dram_tensor("xt", [128, NQ, BL], F16, kind="ExternalInput").ap()
    a1_d = nc.dram_tensor("a1", [128, 64, C], F16, kind="ExternalInput").ap()
    st2_d = nc.dram_tensor("st2", [32, 2, 128], F16, kind="ExternalInput").ap()
    dg_d = nc.dram_tensor("dg", [32, D, C], F16, kind="ExternalInput").ap()
    ob_d = nc.dram_tensor("ob", [1, BL + C], F16, kind="ExternalInput").ap()
    out_d = nc.dram_tensor("probs", [BL, C], F32, kind="ExternalOutput").ap()

    with tile.TileContext(nc) as tc:
        with (
            tc.tile_pool(name="consts", bufs=1) as consts,
            tc.tile_pool(name="xp", bufs=6) as xpool,
            tc.tile_pool(name="fin", bufs=1) as fin,
            tc.tile_pool(name="gps", bufs=2, space="PSUM") as gpspool,
            tc.tile_pool(name="sc", bufs=1, space="PSUM") as scpool,
        ):
            # x DMA stream first so the big transfer starts at t=0
            pre = {}
            for gi in range(2):
                xt = xpool.tile([128, G, BL], F16, tag="xt")
                nc.sync.dma_start(out=xt, in_=xt_d[:, gi * G:(gi + 1) * G, :])
                pre[gi] = xt

            a1 = consts.tile([128, 64, C], F16)
            nc.sync.dma_start(out=a1, in_=a1_d)
            st2 = consts.tile([32, 2, 128], F16)
            nc.sync.dma_start(out=st2, in_=st2_d)
            dg = consts.tile([32, D, C], F16)
            nc.sync.dma_start(out=dg, in_=dg_d)
            ob = consts.tile([1, BL + C], F16)
            nc.sync.dma_start(out=ob, in_=ob_d)

            # ---- W2 dense generation: w2sb[p, nh, m, c] ----
            w2sb = consts.tile([128, 2, D, C], F16)
            for mblk in range(D // MBLK):
                for nh in range(2):
                    gps = gpspool.tile([128, MBLK, C], F32)
                    for j in range(MBLK):
                        m = mblk * MBLK + j
                        nc.tensor.matmul(gps[:, j, :], st2[:, nh, :],
                                         dg[:, m, :], start=True, stop=True)
                    dst = w2sb[:, nh, mblk * MBLK:(mblk + 1) * MBLK, :]
                    if nh == 0:
                        nc.scalar.copy(dst, gps)
                    else:
                        nc.vector.tensor_scalar_add(dst, gps, 0.0)

            # ---- main pass: scores accumulate over all 512 chunks ----
            sc = scpool.tile([BL, C], F32)
            first = True
            for gi in range(NG):
                if gi in pre:
                    xt = pre[gi]
                else:
                    xt = xpool.tile([128, G, BL], F16, tag="xt")
                    nc.sync.dma_start(out=xt,
                                      in_=xt_d[:, gi * G:(gi + 1) * G, :])
                for t in range(G):
                    q = gi * G + t
                    m, nh = q // 2, q % 2
                    nc.tensor.matmul(sc, xt[:, t, :], w2sb[:, nh, m, :],
                                     start=first, stop=False)
                    first = False
                    nc.tensor.matmul(sc, xt[:, t, :], a1[:, m % 64, :],
                                     start=False, stop=False)
            nc.tensor.matmul(sc, ob[:, 0:BL], ob[:, BL:BL + C],
                             start=False, stop=True)

            # ---- softmax over the 16 free elements ----
            negmax = fin.tile([BL, 1], F32)
            nc.vector.tensor_reduce(out=negmax, in_=sc, axis=AXL.X,
                                    op=AOP.max, negate=True)
            e = fin.tile([BL, C], F32)
            sume = fin.tile([BL, 1], F32)
            nc.scalar.activation(out=e, in_=sc, func=AFT.Exp, bias=negmax,
                                 scale=1.0, accum_out=sume)
            rec = fin.tile([BL, 1], F32)
            nc.vector.reciprocal(rec, sume)
            probs = fin.tile([BL, C], F32)
            nc.vector.tensor_scalar_mul(probs, e, rec)
            nc.sync.dma_start(out=out_d, in_=probs)

    nc.compile()
    _cache["nc"] = nc
    return nc


def _host_prep(inputs, w1, w2, l, b):
    inputs = np.asarray(inputs, dtype=np.float32)
    w1 = np.asarray(w1, dtype=np.float32)
    w2 = np.asarray(w2, dtype=np.float32)
    l = np.asarray(l, dtype=np.float32)
    b = np.asarray(b, dtype=np.float32)

    g = np.float32(1.0 / (1.0 + np.exp(-np.float32(l[0]))))

    # g * A1 [c, a, b], duplicated over the 128 partitions (p -> p%64)
    r1 = w1.shape[-1]
    u1, v1 = w1[:, :, r1 // 2:], w1[:, :, :r1 // 2]
    A1 = (np.einsum('car,cbr->cab', u1, u1)
          - np.einsum('car,cbr->cab', v1, v1)) * g
    a1t = np.ascontiguousarray(A1.transpose(2, 1, 0))          # [b, a, c]
    a1_dup = np.concatenate([a1t, a1t], axis=0).astype(np.float16)

    # W2 gen consts: st2[k, n] = (u2|v2)[k, n]; dg[k, m, c] diag-packed
    u2, v2 = w2[:, :, 1], w2[:, :, 0]                          # [16, 256]
    st2 = np.concatenate([u2, v2], axis=0).reshape(32, 2, 128)
    st2 = st2.astype(np.float16)
    dg = np.zeros((32, D, C), np.float32)
    ks = np.arange(16)
    dg[ks, :, ks] = (1.0 - g) * u2
    dg[ks + 16, :, ks] = -(1.0 - g) * v2
    dg = dg.astype(np.float16)

    ob = np.zeros((1, BL + C), np.float32)
    ob[0, :BL] = 1.0
    ob[0, BL:] = b
    ob = ob.astype(np.float16)

    # x: [1024, 256, 256] -> per-core xt[p, q, s] = x[s, q*128 + p]
    xt_all = inputs.astype(np.float16).reshape(N_CORES, BL, NQ, 128)
    xt_all = np.ascontiguousarray(xt_all.transpose(0, 3, 2, 1))

    shared = dict(a1=a1_dup, st2=st2, dg=dg, ob=ob)
    in_maps = []
    for core in range(N_CORES):
        m = dict(shared)
        m["xt"] = xt_all[core]
        in_maps.append(m)
    return in_maps


def kernel(inputs, w1, w2, l, b, _trace=False):
    nc = _build()
    in_maps = _host_prep(inputs, w1, w2, l, b)
    res = run_bass_kernel_spmd(nc, in_maps, core_ids=list(range(N_CORES)),
                               trace=_trace)
    out = np.concatenate([r["probs"] for r in res.results], axis=0)
    if _trace:
        kernel.last_results = res
    return out


# revision 9
# speedup vs baseline: 1.0063x; 1.0063x over previous
"""Trainium2 Bass kernel for nn_CLRBP_23124103922240.

Math: scores[s, c] = x[s] . W[c] + b[c], softmax over 16 classes, where
W[c] = g * tile4x4(A1[c]) + (1-g) * A2[c],
A1[c] = u1 u1^T - v1 v1^T (64x64, rank 8), A2[c] = u2 u2^T - v2 v2^T
(256x256, rank 2), g = sigmoid(l[0]).

Strategy (dense-W, X-stationary):
  - x is cast to fp16 on host (measured output rel-err 4.3e-3, gate 2e-2)
    and re-laid out per core as xt[p, q, s] = x[s, q*128 + p]: the flat
    65536-pixel axis is split into 512 chunks of 128; the DMA stream is
    16.8 MB/core, half of f32.
  - For each chunk q the X block [128 pixels, 128 samples] is the
    *stationary* matmul operand; two [128, 16] moving operands (the g*A1
    table slice and the (1-g)*W2 slice) accumulate scores [128 samples,
    16 classes] directly in PSUM.  No per-sample vector work at all.
  - W1-tiled never materializes: tile4x4 means the moving slice for chunk
    (m, nh) is g*A1[c, m%64, p%64], read from a host-precomputed
    [128, 64, 16] table (p-duplicated so no partition wrap is needed).
  - W2 (dense [65536, 16]) is generated on device from the rank-2 factors:
    for each row m and column half nh, out[p, c] = sum_k st2[k, nh*128+p]
    * dg[k, m, c] with st2 = [u2 | v2] and dg the per-m diag-packed
    (1-g)-scaled factors; 512 tiny matmuls + 16 PSUM->SBUF copies.
  - Bias enters as a K=1 matmul (ones x b); softmax on [128, 16] f32.

Data-parallel over 8 NeuronCores: batch 1024 -> 128 samples per core.
"""

import numpy as np

import concourse.bacc as bacc
import concourse.mybir as mybir
import concourse.tile as tile
from concourse.bass_utils import run_bass_kernel_spmd

N_CORES = 8
B, D, C = 1024, 256, 16
BL = B // N_CORES        # 128 samples per core
NQ = (D * D) // 128      # 512 pixel chunks
G = 16                   # chunks per x DMA group
NG = NQ // G             # 32 groups
MBLK = 32                # W2-gen rows per PSUM bank round

F16 = mybir.dt.float16
F32 = mybir.dt.float32
AOP = mybir.AluOpType
AFT = mybir.ActivationFunctionType
AXL = mybir.AxisListType

_cache = {}


def _build():
    if "nc" in _cache:
        return _cache["nc"]

    nc = bacc.Bacc("TRN2", target_bir_lowering=False, debug=False,
                   num_devices=N_CORES)

    xt_d = nc.dram_tensor("xt", [128, NQ, BL], F16, kind="ExternalInput").ap()
    a1_d = nc.dram_tensor("a1", [128, 64, C], F16, kind="ExternalInput").ap()
    st2_d = nc.dram_tensor("st2", [32, 2, 128], F16, kind="ExternalInput").ap()
    dg_d = nc.dram_tensor("dg", [32, D, C], F16, kind="ExternalInput").ap()
    ob_d = nc.dram_tensor("ob", [1, BL + C], F16, kind="ExternalInput").ap()
    out_d = nc.dram_tensor("probs", [BL, C], F32, kind="ExternalOutput").ap()

    with tile.TileContext(nc) as tc:
        with (
            tc.tile_pool(name="consts", bufs=1) as consts,
            tc.tile_pool(name="xp", bufs=8) as xpool,
            tc.tile_pool(name="fin", bufs=1) as fin,
            tc.tile_pool(name="gps", bufs=2, space="PSUM") as gpspool,
            tc.tile_pool(name="sc", bufs=1, space="PSUM") as scpool,
        ):
            # group schedule: big groups, tapered tail so the end-of-stream
            # drain only covers a couple of chunks
            sizes = [G] * (NQ // G - 1) + [8, 4, 2, 2]
            starts = [sum(sizes[:i]) for i in range(len(sizes))]

            # x DMA stream first so the big transfer starts at t=0; the
            # W2-gen consts (dg, st2) go right behind group 0 so generation
            # starts early
            pre = {}
            xt = xpool.tile([128, sizes[0], BL], F16, tag="xt")
            nc.sync.dma_start(out=xt, in_=xt_d[:, 0:sizes[0], :])
            pre[0] = xt

            dg = consts.tile([32, D, C], F16)
            nc.sync.dma_start(out=dg, in_=dg_d)
            st2 = consts.tile([32, 2, 128], F16)
            nc.sync.dma_start(out=st2, in_=st2_d)

            xt = xpool.tile([128, sizes[1], BL], F16, tag="xt")
            nc.sync.dma_start(out=xt, in_=xt_d[:, starts[1]:starts[1] + sizes[1], :])
            pre[1] = xt

            a1 = consts.tile([128, 64, C], F16)
            nc.sync.dma_start(out=a1, in_=a1_d)
            ob = consts.tile([1, BL + C], F16)
            nc.sync.dma_start(out=ob, in_=ob_d)

            # ---- W2 dense generation: w2sb[p, nh, m, c] ----
            w2sb = consts.tile([128, 2, D, C], F16)
            for mblk in range(D // MBLK):
                for nh in range(2):
                    gps = gpspool.tile([128, MBLK, C], F32)
                    for j in range(MBLK):
                        m = mblk * MBLK + j
                        nc.tensor.matmul(gps[:, j, :], st2[:, nh, :],
                                         dg[:, m, :], start=True, stop=True)
                    dst = w2sb[:, nh, mblk * MBLK:(mblk + 1) * MBLK, :]
                    if nh == 0:
                        nc.scalar.copy(dst, gps)
                    else:
                        nc.vector.tensor_scalar_add(dst, gps, 0.0)

            # ---- main pass: scores accumulate over all 512 chunks ----
            sc = scpool.tile([BL, C], F32)
            first = True
            for gi in range(len(sizes)):
                if gi in pre:
                    xt = pre[gi]
                else:
                    xt = xpool.tile([128, sizes[gi], BL], F16, tag="xt")
                    nc.sync.dma_start(
                        out=xt,
                        in_=xt_d[:, starts[gi]:starts[gi] + sizes[gi], :])
                for t in range(sizes[gi]):
                    q = starts[gi] + t
                    m, nh = q // 2, q % 2
                    nc.tensor.matmul(sc, xt[:, t, :], w2sb[:, nh, m, :],
                                     start=first, stop=False)
                    first = False
                    nc.tensor.matmul(sc, xt[:, t, :], a1[:, m % 64, :],
                                     start=False, stop=False)
            nc.tensor.matmul(sc, ob[:, 0:BL], ob[:, BL:BL + C],
                             start=False, stop=True)

            # ---- softmax over the 16 free elements ----
            negmax = fin.tile([BL, 1], F32)
            nc.vector.tensor_reduce(out=negmax, in_=sc, axis=AXL.X,
                                    op=AOP.max, negate=True)
            e = fin.tile([BL, C], F32)
            sume = fin.tile([BL, 1], F32)
            nc.scalar.activation(out=e, in_=sc, func=AFT.Exp, bias=negmax,
                                 scale=1.0, accum_out=sume)
            rec = fin.tile([BL, 1], F32)
            nc.vector.reciprocal(rec, sume)
            probs = fin.tile([BL, C], F32)
            nc.vector.tensor_scalar_mul(probs, e, rec)
            nc.sync.dma_start(out=out_d, in_=probs)

    nc.compile()
    _cache["nc"] = nc
    return nc


def _host_prep(inputs, w1, w2, l, b):
    inputs = np.asarray(inputs, dtype=np.float32)
    w1 = np.asarray(w1, dtype=np.float32)
    w2 = np.asarray(w2, dtype=np.float32)
    l = np.asarray(l, dtype=np.float32)
    b = np.asarray(b, dtype=np.float32)

    g = np.float32(1.0 / (1.0 + np.exp(-np.float32(l[0]))))

    # g * A1 [c, a, b], duplicated over the 128 partitions (p -> p%64)
    r1 = w1.shape[-1]
    u1, v1 = w1[:, :, r1 // 2:], w1[:, :, :r1 // 2]
    A1 = (np.einsum('car,cbr->cab', u1, u1)
          - np.einsum('car,cbr->cab', v1, v1)) * g
    a1t = np.ascontiguousarray(A1.transpose(2, 1, 0))          # [b, a, c]
    a1t = np.concatenate([a1t, a1t], axis=0).astype(np.float16)

    # W2 gen consts: st2[k, n] = (u2|v2)[k, n]; dg[k, m, c] diag-packed
    u2, v2 = w2[:, :, 1], w2[:, :, 0]                          # [16, 256]
    st2 = np.concatenate([u2, v2], axis=0).reshape(32, 2, 128)
    st2 = st2.astype(np.float16)
    dg = np.zeros((32, D, C), np.float32)
    ks = np.arange(16)
    dg[ks, :, ks] = (1.0 - g) * u2
    dg[ks + 16, :, ks] = -(1.0 - g) * v2
    dg = dg.astype(np.float16)

    ob = np.zeros((1, BL + C), np.float32)
    ob[0, :BL] = 1.0
    ob[0, BL:] = b
    ob = ob.astype(np.float16)

    # x: [1024, 256, 256] -> per-core xt[p, q, s] = x[s, q*128 + p]
    xt_all = inputs.astype(np.float16).reshape(N_CORES, BL, NQ, 128)
    xt_all = np.ascontiguousarray(xt_all.transpose(0, 3, 2, 1))

    shared = dict(a1=a1t, st2=st2, dg=dg, ob=ob)
    in_maps = []
    for core in range(N_CORES):
        m = dict(shared)
        m["xt"] = xt_all[core]
        in_maps.append(m)
    return in_maps


def kernel(inputs, w1, w2, l, b, _trace=False):
    nc = _build()
    in_maps = _host_prep(inputs, w1, w2, l, b)
    res = run_bass_kernel_spmd(nc, in_maps, core_ids=list(range(N_CORES)),
                               trace=_trace)
    out = np.concatenate([r["probs"] for r in res.results], axis=0)
    if _trace:
        kernel.last_results = res
    return out
